# revision 2
# baseline (speedup 1.0000x reference)
"""Dual-branch attention (shared attn weights, se/de value branches) on 8 TRN2 cores.

Sharding: 2 batches x 16 heads = 32 (b,h) pairs; core i owns batch i//4 and
heads [4*(i%4), 4*(i%4)+4) (128 feature channels). Activations are passed
pre-transposed ([C, N]) and in bf16 so the per-core kernel needs no on-chip
transposes. Each core computes its heads' attention for both value branches
and a row-sharded partial of the output projections; the host sums the 4
partials per batch and adds the biases.

v3: scores flow through 2-bank PSUM buffers so exp runs as [128,1024]
activations (amortizes the scalar engine's fixed per-instruction cost);
input DMA is column-split and sequenced (sT before dT) so the k-projection
starts ~6us in; v-projections are branch-split so the se branch never waits
for dT; softmax normalization uses reciprocal_approx_fast and the final
per-head multiplies run on the (otherwise idle) gpsimd engine; transient
PSUM (ps_p) and the PV accumulator (ps_o) are double-buffered.
"""

from contextlib import ExitStack

import numpy as np
import ml_dtypes

import concourse.bass as bass
import concourse.mybir as mybir
import concourse.tile as tile
from concourse import bacc
from concourse.bass import ts, ds
from concourse.bass_utils import run_bass_kernel_spmd

B, N, C, H, D = 2, 2048, 512, 16, 32
SCALE = D ** -0.5
P = 128
CJ = C // P      # 4 contraction chunks for the projections
NJ = 4           # q blocks of 512
KJ = N // NJ     # 512
NK = N // P      # 16 k chunks of 128
HL = 4           # heads per core
F = HL * D       # 128 local feature channels
VW = 2 * D + 1   # per-head vpack width: [v_se | v_de | ones]
NG = 8           # exp chunk-groups per (j,h) unit, 2 chunks each

BF16 = mybir.dt.bfloat16
F32 = mybir.dt.float32
NPBF16 = ml_dtypes.bfloat16


def build_nc():
    nc = bacc.Bacc("TRN2", target_bir_lowering=False, debug=False, num_devices=8)

    sT = nc.dram_tensor("sT", [C, N], BF16, kind="ExternalInput").ap()
    dT = nc.dram_tensor("dT", [C, N], BF16, kind="ExternalInput").ap()
    wq = nc.dram_tensor("wq", [C, F], BF16, kind="ExternalInput").ap()
    wk = nc.dram_tensor("wk", [C, F], BF16, kind="ExternalInput").ap()
    wvs = nc.dram_tensor("wvs", [C, F], BF16, kind="ExternalInput").ap()
    wvd = nc.dram_tensor("wvd", [C, F], BF16, kind="ExternalInput").ap()
    wps = nc.dram_tensor("wps", [F, C], BF16, kind="ExternalInput").ap()
    wpd = nc.dram_tensor("wpd", [F, C], BF16, kind="ExternalInput").ap()
    # packed output: [branch, partition, n-chunk, C] — 8KB-contiguous stores;
    # the host transposes back to [branch, N, C].
    out = nc.dram_tensor("out", [2, P, NK, C], F32, kind="ExternalOutput").ap()

    EXP = mybir.ActivationFunctionType.Exp
    MUL = mybir.AluOpType.mult

    with ExitStack() as ctx:
        tc = ctx.enter_context(tile.TileContext(nc))
        consts = ctx.enter_context(tc.tile_pool(name="consts", bufs=1))
        ppool = ctx.enter_context(tc.tile_pool(name="probs", bufs=18))
        stg = ctx.enter_context(tc.tile_pool(name="stg", bufs=2))
        opool = ctx.enter_context(tc.tile_pool(name="opool", bufs=6))
        rbpool = ctx.enter_context(tc.tile_pool(name="rbpool", bufs=3))
        ps_s = ctx.enter_context(tc.tile_pool(name="ps_s", bufs=2, space="PSUM"))
        ps_o = ctx.enter_context(tc.tile_pool(name="ps_o", bufs=2, space="PSUM"))
        ps_p = ctx.enter_context(tc.tile_pool(name="ps_p", bufs=2, space="PSUM"))

        # ---- q/k storage (emitted first: memsets run while DMA streams) ----
        qtFull = consts.tile([P, N], BF16, tag="qtFull")
        kTz = [consts.tile([P, N], BF16, tag=f"kTz{h}", name=f"kTz{h}")
               for h in range(HL)]
        for h in range(HL):
            nc.vector.memset(kTz[h][:], 0.0)
        ones64 = consts.tile([1, 64], BF16)
        nc.vector.memset(ones64[:], 1.0)

        # ---- loads ----
        # weights first (small, needed first), then sT in column halves (the
        # k-projection can start after the first halves), then dT strictly
        # after — the sync engine issues descriptors in order, so emission
        # order is bandwidth priority.
        wqt = consts.tile([P, CJ, F], BF16, tag="wq")
        wkt = consts.tile([P, CJ, F], BF16, tag="wk")
        wvst = consts.tile([P, CJ, F], BF16, tag="wvs")
        wvdt = consts.tile([P, CJ, F], BF16, tag="wvd")
        for w_ap, w_t in ((wk, wkt), (wq, wqt)):
            nc.sync.dma_start(w_t[:], w_ap.rearrange("(co p) f -> p co f", p=P))
        sT3 = sT.rearrange("(co p) n -> p co n", p=P)
        dT3 = dT.rearrange("(co p) n -> p co n", p=P)
        sTc = [consts.tile([P, N], BF16, tag=f"sT{c}", name=f"sT{c}")
               for c in range(CJ)]
        dTc = [consts.tile([P, N], BF16, tag=f"dT{c}", name=f"dT{c}")
               for c in range(CJ)]
        for half in range(2):
            for c in range(CJ):
                nc.sync.dma_start(
                    sTc[c][:, ts(half, N // 2)], sT3[:, c, ts(half, N // 2)])
        for w_ap, w_t in ((wvs, wvst), (wvd, wvdt)):
            nc.sync.dma_start(w_t[:], w_ap.rearrange("(co p) f -> p co f", p=P))
        for half in range(2):
            for c in range(CJ):
                nc.sync.dma_start(
                    dTc[c][:, ts(half, N // 2)], dT3[:, c, ts(half, N // 2)])
        wpst = consts.tile([P, C], BF16, tag="wps")
        wpdt = consts.tile([P, C], BF16, tag="wpd")
        nc.sync.dma_start(wpst[:], wps)
        nc.sync.dma_start(wpdt[:], wpd)

        def emit_qproj(j):
            ps = ps_p.tile([P, KJ], F32, tag="pp", name="pp_qk")
            for c in range(CJ):
                nc.tensor.matmul(
                    ps[:], wqt[:, c], sTc[c][:, ts(j, KJ)],
                    start=(c == 0), stop=(c == CJ - 1),
                )
            nc.vector.tensor_copy(qtFull[:, ts(j, KJ)], ps[:])

        # ---- value projections, natural [N, feat] layout, packed per head ----
        vpk = [consts.tile([P, HL * VW], BF16, tag=f"vpk{n}", name=f"vpk{n}")
               for n in range(NK)]
        for n in range(NK):
            nc.vector.memset(
                vpk[n].rearrange("p (h y) -> p h y", h=HL)[:, :, 2 * D:2 * D + 1],
                1.0)

        def emit_vproj(n, br):
            act, w_t = ((sTc, wvst), (dTc, wvdt))[br]
            ps = ps_p.tile([P, KJ], F32, tag="pp", name="pp_v")
            for c in range(CJ):
                nc.tensor.matmul(
                    ps[:, :F], act[c][:, ts(n, P)], w_t[:, c],
                    start=(c == 0), stop=(c == CJ - 1),
                )
            dst = vpk[n].rearrange("p (h y) -> p h y", h=HL)[:, :, br * D:(br + 1) * D]
            src = ps[:, :F].rearrange("p (h d) -> p h d", h=HL)
            nc.vector.tensor_copy(dst, src)

        # ---- attention ----
        outTs = consts.tile([P, N], BF16, tag="oTs")
        outTd = consts.tile([P, N], BF16, tag="oTd")

        jstate = {}

        def emit_pv_tail(j, h, op):
            """Stage the PV result + its sums row in SBUF (frees the PSUM
            bank; sums of all 4 heads batch into one reciprocal per block)."""
            if j not in jstate:
                rb4 = stg.tile([P, KJ], F32, tag="rb4", name="rb4")
                nc.vector.memset(rb4[:], 1.0)
                jstate[j] = (rb4, {})
            rb4, opcs = jstate[j]
            nc.vector.tensor_copy(rb4[ds(h * D, 1), :], op[64:65, :])
            opc = opool.tile([64, KJ], F32, tag="opc", name=f"opc{h}")
            nc.vector.tensor_copy(opc[:], op[0:64, :])
            opcs[h] = opc

        def emit_jtail(j):
            rb4, opcs = jstate.pop(j)
            rcp4 = stg.tile([P, KJ], F32, tag="rcp4")
            nc.vector.reciprocal_approx_fast(rcp4[:], rb4[:])
            for h in range(HL):
                rsb = stg.tile([1, KJ], BF16, tag="rsb")
                nc.vector.tensor_copy(rsb[:], rcp4[ds(h * D, 1), :])
                rb = ps_p.tile([64, KJ], F32, tag="pp", name="pp_rb")
                nc.tensor.matmul(rb[:], ones64[:], rsb[:], start=True, stop=True)
                rbb = rbpool.tile([64, KJ], F32, tag="rbb")
                nc.vector.tensor_copy(rbb[:], rb[:])
                opc = opcs[h]
                nc.gpsimd.tensor_tensor(
                    outTs[ds(h * D, D), ts(j, KJ)], opc[0:D, :], rbb[0:D, :], MUL)
                nc.gpsimd.tensor_tensor(
                    outTd[ds(h * D, D), ts(j, KJ)], opc[D:2 * D, :], rbb[D:2 * D, :], MUL)

        ostate = {}

        def emit_outproj_piece(j, nn):
            """One output chunk (both branches) — spread across iterations so
            the out-projection never blocks the QK stream for long."""
            for br, (oT, wp_t) in enumerate(((outTs, wpst), (outTd, wpdt))):
                if (j, br) not in ostate:
                    ostate[(j, br)] = stg.tile(
                        [P, NJ, KJ], F32, tag="st", name=f"st{br}")
                st = ostate[(j, br)]
                pp = ps_p.tile([P, KJ], F32, tag="pp", name="pp_o")
                nc.tensor.matmul(
                    pp[:], oT[:, ds((NJ * j + nn) * P, P)], wp_t[:],
                    start=True, stop=True,
                )
                nc.vector.tensor_copy(st[:, nn], pp[:])
                if nn == NJ // 2 - 1:
                    nc.sync.dma_start(
                        out[br][:, ds(NJ * j, NJ // 2)], st[:, 0:NJ // 2])
                if nn == NJ - 1:
                    nc.sync.dma_start(
                        out[br][:, ds(NJ * j + NJ // 2, NJ // 2)],
                        st[:, NJ // 2:NJ])
                    del ostate[(j, br)]

        # ---- prologue: k-projection (all blocks) + q-projection (block 0) ----
        # c-OUTER so each activation chunk is consumed as soon as its DMA
        # lands. Accumulators live in the (not-yet-needed) scores buffers.
        kpsA = ps_s.tile([P, 2, KJ], F32, tag="sc2", name="kpsA")
        kpsB = ps_s.tile([P, 2, KJ], F32, tag="sc2", name="kpsB")
        kps = [kpsA[:, 0, :], kpsA[:, 1, :], kpsB[:, 0, :], kpsB[:, 1, :]]
        q0ps = ps_o.tile([P, KJ], F32, tag="op", name="q0ps")
        for c in range(CJ):
            for j in range(NJ):
                nc.tensor.matmul(
                    kps[j], wkt[:, c], sTc[c][:, ts(j, KJ)],
                    start=(c == 0), stop=(c == CJ - 1),
                )
            nc.tensor.matmul(
                q0ps[:], wqt[:, c], sTc[c][:, ts(0, KJ)],
                start=(c == 0), stop=(c == CJ - 1),
            )
        nc.vector.tensor_copy(qtFull[:, ts(0, KJ)], q0ps[:])
        for h in range(HL):      # h-outer: head 0's kTz completes first
            for j in range(NJ):
                nc.vector.tensor_copy(
                    kTz[h][ds(h * D, D), ts(j, KJ)], kps[j][ds(h * D, D), :])

        # ---- main loop ----
        # Per (j,h) unit: 8 windows of [2 QK matmuls -> exp -> 2 PV chunks of
        # the previous unit -> interleaved projection work].
        vq_se = list(range(NK))
        vq_de = list(range(NK))
        ojobs = []
        prev, prs_prev = None, None
        for j in range(NJ):
            for h in range(HL):
                prs_groups = []
                pv_m = 0
                op = None
                if prev is not None:
                    op = ps_o.tile([P, KJ], F32, tag="op", name="op")
                for gi in range(NG):
                    sc = ps_s.tile([P, 2, KJ], F32, tag="sc2", name="sc")
                    for i in range(2):
                        m = 2 * gi + i
                        nc.tensor.matmul(
                            sc[:, i, :], kTz[h][:, ts(m, P)],
                            qtFull[:, ts(j, KJ)], start=True, stop=True)
                    prg = ppool.tile([P, 2, KJ], BF16, tag="pr2", name="prg")
                    nc.scalar.activation(prg[:], sc[:], EXP, scale=SCALE)
                    prs_groups.append(prg)
                    # de-branch v-projections (must precede the matching PV
                    # chunk of the previous unit; dT lands after sT)
                    if prev == (0, 0):
                        for _ in range(3):
                            if vq_de:
                                emit_vproj(vq_de.pop(0), 1)
                    # PV of the previous unit
                    if prev is not None:
                        for _ in range(2):
                            if pv_m < NK:
                                pg, pi = divmod(pv_m, 2)
                                nc.tensor.matmul(
                                    op[:VW, :],
                                    vpk[pv_m][:, ds(prev[1] * VW, VW)],
                                    prs_prev[pg][:, pi, :],
                                    start=(pv_m == 0), stop=(pv_m == NK - 1),
                                )
                                pv_m += 1
                    # se-branch v-projections (early units only)
                    if vq_se and (j, h) == (0, 0):
                        for _ in range(2):
                            if vq_se:
                                emit_vproj(vq_se.pop(0), 0)
                    elif vq_se:
                        emit_vproj(vq_se.pop(0), 0)
                    if gi == 4 and h == 1 and j < NJ - 1:
                        emit_qproj(j + 1)
                    if gi in (1, 3, 5) and ojobs:
                        emit_outproj_piece(*ojobs.pop(0))
                if prev is not None:
                    emit_pv_tail(*prev, op)
                    if prev[1] == HL - 1:
                        emit_jtail(prev[0])
                        ojobs += [(prev[0], nn) for nn in range(NJ)]
                prev, prs_prev = (j, h), prs_groups
        # ---- tail ----
        op = ps_o.tile([P, KJ], F32, tag="op", name="op")
        for pv_m in range(NK):
            pg, pi = divmod(pv_m, 2)
            nc.tensor.matmul(
                op[:VW, :], vpk[pv_m][:, ds(prev[1] * VW, VW)],
                prs_prev[pg][:, pi, :],
                start=(pv_m == 0), stop=(pv_m == NK - 1),
            )
        emit_pv_tail(*prev, op)
        emit_jtail(prev[0])
        ojobs += [(prev[0], nn) for nn in range(NJ)]
        for jb in ojobs:
            emit_outproj_piece(*jb)

    nc.compile()
    return nc


_NC_CACHE = {}


def _get_nc():
    if "nc" not in _NC_CACHE:
        _NC_CACHE["nc"] = build_nc()
    return _NC_CACHE["nc"]


def make_in_maps(se, de, W_qkv_se, W_v_de, W_proj_se, W_proj_de):
    se = np.asarray(se, dtype=np.float32)
    de = np.asarray(de, dtype=np.float32)
    W_qkv_se = np.asarray(W_qkv_se, dtype=np.float32)
    W_v_de = np.asarray(W_v_de, dtype=np.float32)
    W_proj_se = np.asarray(W_proj_se, dtype=np.float32)
    W_proj_de = np.asarray(W_proj_de, dtype=np.float32)
    qW, kW, vW = W_qkv_se[:, 0:C], W_qkv_se[:, C:2 * C], W_qkv_se[:, 2 * C:3 * C]

    sTs = [np.ascontiguousarray(se[b].T).astype(NPBF16) for b in range(B)]
    dTs = [np.ascontiguousarray(de[b].T).astype(NPBF16) for b in range(B)]
    in_maps = []
    for core in range(8):
        b, g = divmod(core, 4)
        sl = slice(g * F, (g + 1) * F)
        in_maps.append({
            "sT": sTs[b],
            "dT": dTs[b],
            "wq": np.ascontiguousarray(qW[:, sl]).astype(NPBF16),
            "wk": np.ascontiguousarray(kW[:, sl]).astype(NPBF16),
            "wvs": np.ascontiguousarray(vW[:, sl]).astype(NPBF16),
            "wvd": np.ascontiguousarray(W_v_de[:, sl]).astype(NPBF16),
            "wps": np.ascontiguousarray(W_proj_se[sl, :]).astype(NPBF16),
            "wpd": np.ascontiguousarray(W_proj_de[sl, :]).astype(NPBF16),
        })
    return in_maps


def gather_out(outs, b_proj_se, b_proj_de):
    b_proj_se = np.asarray(b_proj_se, dtype=np.float32)
    b_proj_de = np.asarray(b_proj_de, dtype=np.float32)
    # per-core out is packed [branch, partition, n-chunk, C]
    outs = [o.transpose(0, 2, 1, 3).reshape(2, N, C) for o in outs]
    out_se = np.stack(
        [sum(outs[4 * b + g][0] for g in range(4)) for b in range(B)]
    ) + b_proj_se[None, None, :]
    out_de = np.stack(
        [sum(outs[4 * b + g][1] for g in range(4)) for b in range(B)]
    ) + b_proj_de[None, None, :]
    return out_se.astype(np.float32), out_de.astype(np.float32)


def kernel(se, de, W_qkv_se, W_v_de, W_proj_se, b_proj_se, W_proj_de, b_proj_de):
    nc = _get_nc()
    in_maps = make_in_maps(se, de, W_qkv_se, W_v_de, W_proj_se, W_proj_de)
    res = run_bass_kernel_spmd(nc, in_maps, core_ids=list(range(8)))
    outs = [r["out"] for r in res.results]
    return gather_out(outs, b_proj_se, b_proj_de)
